# revision 81
# baseline (speedup 1.0000x reference)
"""HRR adapted attention kernel for 8 trn2 cores — frequency-sharded.

Math (verified in numpy):
  q,k,v = h @ W{q,k,v}.T + b      (per-row, D=2048)
  Qf = rfft(q); Kf = rfft(k)/(|rfft(k)|+eps); Vf likewise
  Mf = causal-cumsum_S(Kf*Vf);  Of = conj(Qf)*Mf;  adapter = irfft(Of)
  out = base + gate*adapter

Sharding: each core owns 128 of the 1024 packed rfft bins and processes
ALL B*S=8192 rows for those bins.  The DFT-folded projections
G = W.T @ C_slice are computed on the HOST (exact, via rfft of W.T) and
shipped as a two-term fp8 split G = G1 + G2 (G2 the quantization
residual); h likewise as h1 + h2, pre-transposed to [D, R].  The device
projection runs three fp8 DoubleRow matmul phases
(h1G1 + h1G2 + h2G1, dropping the negligible h2G2), which is both
faster than bf16 (DoubleRow contracts 256 per instruction at half
cycles/row) and slightly more accurate.  The causal scan runs fully
on-core with tensor_tensor_scan carry chaining.  The only collective is
a per-block ReduceScatter of the bf16 irfft partial sums, writing the
gate-scaled adapter directly to the output; base is added on the host
in f32.  Blocks are [4,4,2,2,1,1,2] chunks so the late collectives
pipeline at chunk rate and the drain tail is one small RS (the final
two chunks share one collective — their irffts complete too close
together for separate ones to pipeline).  Collectives get the gpsimd
queue to themselves (their input waits block the queue head); only the
last chunk's bind chain borrows Pool for its im channel, when the Pool
queue is clear of pending collective waits.

Packed spectrum: row 0 re-plane = DC, row 0 im-plane = Nyquist (both
real bins); their rank-1 contribution is added on the host.
"""

import numpy as np
import ml_dtypes

import concourse.bass as bass
import concourse.mybir as mybir
import concourse.tile as tile
from concourse import bacc, bass_utils

F32 = mybir.dt.float32
BF16 = mybir.dt.bfloat16
F8 = mybir.dt.float8e4
ALU = mybir.AluOpType
ACTF = mybir.ActivationFunctionType
DROW = mybir.MatmulPerfMode.DoubleRow

B, S, D = 2, 4096, 2048
R = B * S                  # 8192 flat rows
NCORES = 8
FP = D // 2                # 1024 packed rfft bins
FPC = FP // NCORES         # 128 bins per core
CH = 512                   # rows per chunk
NCH = R // CH              # 16 chunks
NE = D // 128              # 16 contraction tiles
NE2 = NE // 2              # 8 DoubleRow contraction tiles
DC = D // NCORES           # 256 output d-columns per core
EPS = 1e-8
BF = ml_dtypes.bfloat16
F8NP = ml_dtypes.float8_e4m3fn

# ReduceScatter blocks (in chunks): big early blocks fully overlap
# compute; per-chunk blocks at the end pipeline at chunk rate so the
# drain tail is a single small collective.
BLOCK_SIZES = [4, 4, 2, 2, 1, 1, 2]
BLOCK_ENDS = list(np.cumsum(BLOCK_SIZES) - 1)
BLOCK_STARTS = [e - s + 1 for e, s in zip(BLOCK_ENDS, BLOCK_SIZES)]
CHUNK_BLOCK = {}
for _b, (_s, _e) in enumerate(zip(BLOCK_STARTS, BLOCK_ENDS)):
    for _c in range(_s, _e + 1):
        CHUNK_BLOCK[_c] = _b
NBLK = len(BLOCK_SIZES)
S_O = 2.0 ** -7            # spectrum prescale for the fp8 irfft split

_CACHE = {}


def _build():
    nc = bacc.Bacc("TRN2", target_bir_lowering=False, debug=False,
                   enable_asserts=False, num_devices=NCORES)

    h_ins = [nc.dram_tensor(f"h{i}", [D, R], F8, kind="ExternalInput").ap()
             for i in (1, 2)]
    g_ins = [[nc.dram_tensor(f"g{i}{x}", [D, 2 * FPC], F8,
                             kind="ExternalInput").ap() for x in "kvq"]
             for i in (1, 2)]
    ab_ins = [nc.dram_tensor(f"ab{i}", [FPC, 2 * D], F8,
                             kind="ExternalInput").ap() for i in (1, 2)]
    dsc_in = nc.dram_tensor("dsc", [128, 1], F32, kind="ExternalInput").ap()
    bf_in = nc.dram_tensor("bfc", [FPC, 6], F32, kind="ExternalInput").ap()
    out_t = nc.dram_tensor("out", [DC, R], BF16, kind="ExternalOutput").ap()

    with tile.TileContext(nc) as tc, \
         tc.tile_pool(name="pc", bufs=1) as PC, \
         tc.tile_pool(name="pm", bufs=2) as PM, \
         tc.tile_pool(name="pt", bufs=1) as PT, \
         tc.tile_pool(name="pev", bufs=3) as PEV, \
         tc.tile_pool(name="psum", bufs=1, space="PSUM") as PP, \
         tc.tile_pool(name="dram", bufs=1, space="DRAM") as DR:

        state = {}

        NEH = NE // 2          # t-tiles per h half-tile

        def load_h_half(i, c, half, eng=None):
            r0 = c * CH
            hT = PM.tile([128, NEH * CH], F8, tag=f"hT{i}{half}",
                         name=f"hT{i}{half}", bufs=3)
            (eng or nc.sync).dma_start(
                hT[:].rearrange("p (t s) -> p t s", t=NEH),
                h_ins[i - 1][:, r0:r0 + CH]
                .rearrange("(t p) s -> p t s", p=128)
                [:, half * NEH:(half + 1) * NEH, :])
            state[(f"h{i}{half}", c)] = hT

        def load_hT(c):
            for i in (1, 2):
                for half in range(2):
                    load_h_half(i, c, half)

        def load_g(eng, i, w, split=1):
            g_sb = PC.tile([128, NE * 2 * FPC], F8, tag=f"G{i}{w}",
                           name=f"G{i}{w}")
            tq = NE // split
            for q in range(split):
                eng.dma_start(
                    g_sb[:].rearrange("p (t f) -> p t f", t=NE)
                    [:, q * tq:(q + 1) * tq, :],
                    g_ins[i - 1][w].rearrange("(t p) f -> p t f", p=128)
                    [:, q * tq:(q + 1) * tq, :])
            return g_sb

        # ---------- startup loads, dependency-ordered on the sync queue
        # (the scalar queue's act-table loads would delay them); the proj
        # phase order is (h1G1, h2G1, h1G2) so the G2 terms come last.
        bf_sb = PC.tile([128, 6], F32, tag="bf")
        nc.gpsimd.dma_start(bf_sb[:], bf_in[:])
        G1 = [None] * 3
        G2 = [None] * 3
        load_h_half(1, 0, 0)
        G1[0] = load_g(nc.sync, 1, 0, split=2)
        load_h_half(1, 0, 1)
        load_h_half(2, 0, 0)
        load_h_half(2, 0, 1)
        G2[0] = load_g(nc.sync, 2, 0)
        G1[1] = load_g(nc.sync, 1, 1)
        G2[1] = load_g(nc.sync, 2, 1)
        G1[2] = load_g(nc.sync, 1, 2)
        G2[2] = load_g(nc.sync, 2, 2)
        load_hT(1)
        eps_sb = PC.tile([128, 1], F32, tag="eps")
        nc.vector.memset(eps_sb[:], EPS * EPS)
        zeros_sb = PC.tile([128, CH], F32, tag="zeros")
        nc.vector.memset(zeros_sb[:], 0.0)
        ab_sb = []
        for i in (1, 2):
            t = PC.tile([128, 2 * D], F8, tag=f"ab{i}")
            nc.sync.dma_start(t[:], ab_ins[i - 1][:])
            ab_sb.append(t)
        dsc_sb = PC.tile([128, 1], F32, tag="dsc")
        nc.sync.dma_start(dsc_sb[:], dsc_in[:])
        PRE_HT = 2

        # ---------- DRAM intermediates ----------
        part = [DR.tile([D, BLOCK_SIZES[b] * CH], BF16, tag=f"part{b}",
                        name=f"part{b}") for b in range(NBLK)]
        rsout = [DR.tile([DC, BLOCK_SIZES[b] * CH], BF16, tag=f"rso{b}",
                         name=f"rso{b}") for b in range(NBLK)]

        def proj_bind(c):
            hh = {(i, half): state.pop((f"h{i}{half}", c))
                  for i in (1, 2) for half in range(2)}
            phases = ((1, G1), (2, G1), (1, G2))
            planes = []
            for mi in range(6):
                w, hf = mi // 2, mi % 2
                ps = PP.tile([128, CH], F32, tag=f"pp{mi % 2}", name="pp",
                             bufs=2)
                for pi, (hi, gsrc) in enumerate(phases):
                    g3 = gsrc[w][:].rearrange("p (t f) -> p t f", t=NE)
                    for e2 in range(NE2):
                        h3 = hh[(hi, e2 // 4)][:].rearrange(
                            "p (t s) -> p t s", t=NEH)
                        eh = e2 % 4
                        nc.tensor.matmul(
                            ps[:],
                            g3[:, 2 * e2:2 * e2 + 2,
                               hf * FPC:(hf + 1) * FPC],
                            h3[:, 2 * eh:2 * eh + 2, :],
                            start=(pi == 0 and e2 == 0),
                            stop=(pi == 2 and e2 == NE2 - 1),
                            perf_mode=DROW)
                pl = PM.tile([128, CH], BF16 if mi >= 4 else F32,
                             tag=f"pl{mi}", name=f"pl{mi}")
                if mi in (1, 3):
                    nc.vector.tensor_scalar_add(pl[:], ps[:],
                                                bf_sb[:, mi:mi + 1])
                else:
                    # q planes (mi 4,5) are pre-scaled by S_O for the fp8
                    # spectrum split; their bfc bias is host-prescaled.
                    nc.scalar.activation(pl[:], ps[:], ACTF.Identity,
                                         bias=bf_sb[:, mi:mi + 1],
                                         scale=S_O if mi >= 4 else 1.0)
                planes.append(pl)
            kre, kim, vre, vim, qre, qim = planes

            def T(tg):
                return PT.tile([128, CH], F32, tag=tg, name=tg)

            # On the LAST chunk the bind chain is the drain critical path
            # (nothing overlaps it): run the im channel on the idle Pool
            # engine in parallel with the re channel on DVE.  Mid-stream,
            # everything stays on DVE (Pool's queue stalls on collective
            # input waits).
            last = c == NCH - 1
            IM = nc.gpsimd if last else nc.vector

            t1, t2 = T("t1"), T("t2")
            rk, rv = T("rk"), T("rv")
            nc.scalar.square(t1[:], kre[:])
            nc.scalar.square(t2[:], kim[:])
            nc.vector.tensor_add(rk[:], t1[:], t2[:])
            nc.scalar.square(t1[:], vre[:])
            nc.scalar.square(t2[:], vim[:])
            nc.vector.tensor_add(rv[:], t1[:], t2[:])
            nc.vector.tensor_mul(rk[:], rk[:], rv[:])
            nc.scalar.activation(rk[:], rk[:], ACTF.Sqrt, bias=eps_sb[:])
            nc.vector.reciprocal(rk[:], rk[:])
            cre, cim = T("cre"), T("cim")
            ti1, ti2 = T("ti1"), T("ti2")
            nc.vector.tensor_mul(t1[:], kre[:], vre[:])
            nc.vector.tensor_mul(t2[:], kim[:], vim[:])
            nc.vector.tensor_sub(cre[:], t1[:], t2[:])
            IM.tensor_mul(ti1[:], kre[:], vim[:])
            IM.tensor_mul(ti2[:], kim[:], vre[:])
            IM.tensor_add(cim[:], ti1[:], ti2[:])
            nc.vector.tensor_mul(cre[:], cre[:], rk[:])
            IM.tensor_mul(cim[:], cim[:], rk[:])
            # causal scan (f32 accumulate, bf16 store); carry chains
            # across chunks, resets per batch
            mre = PM.tile([128, CH], F32, tag="mre", name="mre")
            mim = PM.tile([128, CH], F32, tag="mim", name="mim")
            mreb = PM.tile([128, CH], BF16, tag="mreb", name="mreb")
            mimb = PM.tile([128, CH], BF16, tag="mimb", name="mimb")
            if c % (NCH // B) == 0:
                ire, iim = 0.0, 0.0
            else:
                pmre, pmim = state["m"]
                ire, iim = pmre[:, CH - 1:CH], pmim[:, CH - 1:CH]
            nc.vector.tensor_tensor_scan(mre[:], cre[:], zeros_sb[:], ire,
                                         ALU.add, ALU.add)
            nc.vector.tensor_tensor_scan(mim[:], cim[:], zeros_sb[:], iim,
                                         ALU.add, ALU.add)
            state["m"] = (mre, mim)
            nc.scalar.copy(mreb[:], mre[:])
            nc.scalar.copy(mimb[:], mim[:])
            # unbind (of = conj(q)*m, all-bf16, 2x DVE) + two-term fp8
            # split of the S_O-scaled spectrum; re and im sit adjacent in
            # one tile so the DoubleRow irfft contracts both in a single
            # instruction per d-tile.
            u1 = PT.tile([128, CH], BF16, tag="u1", name="u1")
            u2 = PT.tile([128, CH], BF16, tag="u2", name="u2")
            ui1 = PT.tile([128, CH], BF16, tag="ui1", name="ui1")
            ui2 = PT.tile([128, CH], BF16, tag="ui2", name="ui2")
            orf = PM.tile([128, CH], BF16, tag="orf", name="orf")
            oif = PM.tile([128, CH], BF16, tag="oif", name="oif")
            nc.vector.tensor_mul(u1[:], qre[:], mreb[:])
            nc.vector.tensor_mul(u2[:], qim[:], mimb[:])
            nc.vector.tensor_add(orf[:], u1[:], u2[:])
            IM.tensor_mul(ui1[:], qre[:], mimb[:])
            IM.tensor_mul(ui2[:], qim[:], mreb[:])
            # the dependent im tail stays on DVE even on the last chunk:
            # Pool is ~3x slower per op and would serialize the drain
            nc.vector.tensor_sub(oif[:], ui1[:], ui2[:])
            s1 = PM.tile([128, 2 * CH], F8, tag="s1", name="s1")
            s2 = PM.tile([128, 2 * CH], F8, tag="s2", name="s2")
            dsr = PT.tile([128, CH], BF16, tag="dsr", name="dsr")
            dsi = PT.tile([128, CH], BF16, tag="dsi", name="dsi")
            cp = nc.vector.tensor_copy if last else nc.scalar.copy
            cp(s1[:, :CH], orf[:])
            nc.vector.tensor_sub(dsr[:], orf[:], s1[:, :CH])
            cp(s2[:, :CH], dsr[:])
            cp(s1[:, CH:], oif[:])
            nc.vector.tensor_sub(dsi[:], oif[:], s1[:, CH:])
            cp(s2[:, CH:], dsi[:])
            state[("of", c)] = [(s1, s2, 0, CH)]

        def irfft_rs(c):
            b = CHUNK_BLOCK[c]
            ci = c - BLOCK_STARTS[b]
            ab3 = [t[:].rearrange("p (t two f) -> p t two f", t=NE, two=2)
                   for t in ab_sb]
            last = c == NCH - 1
            JG = 4 if last else 8     # j-tiles per staging store; the last
            # chunk stores quarter-granular so its final part write (and
            # the final collective) launches sooner.
            if last:
                # dependency-free filler matmuls: keep the PE p-state hot
                # across the bind-chain gap so the final irfft runs at
                # full rate the moment its operands land (the cost model
                # halves matmul speed for ~3us after any idle gap).
                fill = PP.tile([128, CH], F32, tag="pp0", name="fill",
                               bufs=2)
                for _ in range(50):
                    nc.tensor.matmul(fill[:], ab_sb[0][:, :128],
                                     ab_sb[0][:, :CH],
                                     start=True, stop=True)
            for s1, s2, col0, wd in state.pop(("of", c)):
                s13 = s1[:].rearrange("p (t s) -> p t s", t=2)
                s23 = s2[:].rearrange("p (t s) -> p t s", t=2)
                for half in range(2):
                    for grp in range(8 // JG):
                        stg = PEV.tile([128, 8 * CH], BF16,
                                       tag=f"pstg{half}",
                                       name=f"pstg{half}", bufs=3)
                        for jj in range(JG):
                            j = grp * JG + jj
                            dt = half * 8 + j
                            pi = PP.tile([128, CH], F32, tag="pirf",
                                         name="pirf", bufs=4)
                            nc.tensor.matmul(pi[:, :wd], ab3[0][:, dt], s13,
                                             start=True, stop=False,
                                             perf_mode=DROW)
                            nc.tensor.matmul(pi[:, :wd], ab3[1][:, dt], s13,
                                             start=False, stop=False,
                                             perf_mode=DROW)
                            nc.tensor.matmul(pi[:, :wd], ab3[0][:, dt], s23,
                                             start=False, stop=True,
                                             perf_mode=DROW)
                            dst = stg[:, jj * wd:(jj + 1) * wd]
                            if dt % 2 == 0:
                                nc.vector.tensor_scalar_mul(dst, pi[:, :wd],
                                                            dsc_sb[:])
                            else:
                                nc.scalar.activation(dst, pi[:, :wd],
                                                     ACTF.Identity,
                                                     scale=dsc_sb[:])
                        r0 = half * 8 * 128 + grp * JG * 128
                        nc.scalar.dma_start(
                            part[b][r0:r0 + JG * 128,
                                    ci * CH + col0:ci * CH + col0 + wd]
                            .rearrange("(t p) s -> p t s", p=128),
                            stg[:, :JG * wd]
                            .rearrange("p (t s) -> p t s", t=JG))
            if c == BLOCK_ENDS[b]:
                r0 = BLOCK_STARTS[b] * CH
                r1 = (BLOCK_ENDS[b] + 1) * CH
                nc.gpsimd.collective_compute(
                    "ReduceScatter", ALU.add,
                    replica_groups=[list(range(NCORES))],
                    ins=[part[b].opt()], outs=[rsout[b].opt()])
                nc.sync.dma_start(out_t[:, r0:r1], rsout[b][:])

        for it in range(NCH + 2):
            if it + PRE_HT < NCH:
                load_hT(it + PRE_HT)
            if it < NCH:
                proj_bind(it)
            if 1 <= it <= NCH:
                irfft_rs(it - 1)

    nc.compile()
    return nc


def _constants():
    """Cached: per-core irfft matrices am/bm (f64)."""
    e = np.arange(D, dtype=np.float64)
    ams, bms = [], []
    for c in range(NCORES):
        js = np.arange(c * FPC, (c + 1) * FPC, dtype=np.float64)
        am = (2.0 / D) * np.cos(2.0 * np.pi * np.outer(js, e) / D)
        bm = -(2.0 / D) * np.sin(2.0 * np.pi * np.outer(js, e) / D)
        if c == 0:
            am[0, :] = 0.0
            bm[0, :] = 0.0
        ams.append(am)
        bms.append(bm)
    return ams, bms


def _two_term_fp8(x):
    x1 = x.astype(F8NP)
    x2 = (x - x1.astype(x.dtype)).astype(F8NP)
    return x1, x2


def _run(inputs, trace=False):
    if "nc" not in _CACHE:
        _CACHE["nc"] = _build()
    nc = _CACHE["nc"]
    ams, bms = _CACHE.setdefault("const", _constants())

    h32 = np.asarray(inputs["hidden_states"], np.float32).reshape(R, D)
    hT = np.ascontiguousarray(h32.T)                     # [D, R] f32
    h1, h2 = _two_term_fp8(hT)
    base = np.asarray(inputs["base_output"], np.float32).reshape(R, D)
    gate = np.asarray(inputs["gate"], np.float32).reshape(-1)[0]

    # Host-side fold of the DFT into the projections via rfft of W.T
    # (exact):  G[:, core c bins] = [Re(F[:, js]) | Im(F[:, js])] where
    # F = rfft(W.T, axis=1); Im comes from the -sin convention.
    gf1, gf2 = {}, {}
    for x in "kvq":
        w = np.asarray(inputs[f"W{x}"], np.float64)
        F = np.fft.rfft(w.T, axis=1)                     # [D, FP+1]
        g = np.empty((D, 2 * FP), np.float64)
        blocks = g.reshape(D, NCORES, 2, FPC)
        Fre = F.real[:, :FP].reshape(D, NCORES, FPC)
        Fim = F.imag[:, :FP].reshape(D, NCORES, FPC)
        blocks[:, :, 0, :] = Fre
        blocks[:, :, 1, :] = Fim
        blocks[:, 0, 0, 0] = 0.0                         # DC re
        blocks[:, 0, 1, 0] = 0.0                         # packed Nyquist slot
        gf1[x], gf2[x] = _two_term_fp8(g)

    bfc = np.zeros((FP, 6), np.float64)
    for j, bn in enumerate(("bk", "bv", "bq")):
        spec = np.fft.rfft(np.asarray(inputs[bn], np.float64))
        bfc[:, 2 * j] = spec.real[:FP]
        bfc[:, 2 * j + 1] = spec.imag[:FP]
        bfc[0, 2 * j] = 0.0
        bfc[0, 2 * j + 1] = 0.0
    bfc[:, 4:6] *= S_O        # q-plane bias prescale (matches scale=S_O)
    bfc = bfc.astype(np.float32)

    # irfft matrices, gate-folded, scaled into the fp8 sweet spot and
    # packed [FPC, NE, (a|b), 128] for DoubleRow stationaries.
    gmax = max(abs(float(gate)), 1e-30) * (2.0 / D)
    s_a = 2.0 ** np.floor(np.log2(200.0 / gmax))
    descale = np.full((128, 1), 1.0 / (s_a * S_O), np.float32)
    ab1s, ab2s = [], []
    for c in range(NCORES):
        ab = np.empty((FPC, NE, 2, 128), np.float64)
        ab[:, :, 0, :] = (ams[c] * (gate * s_a)).reshape(FPC, NE, 128)
        ab[:, :, 1, :] = (bms[c] * (gate * s_a)).reshape(FPC, NE, 128)
        a1, a2 = _two_term_fp8(ab.reshape(FPC, 2 * D))
        ab1s.append(a1)
        ab2s.append(a2)

    # Exact host-side handling of the two real bins (DC, Nyquist): their
    # adapter contribution is rank-1 over d; added on the host.
    h64 = np.asarray(inputs["hidden_states"], np.float64).reshape(R, D)
    sgn = np.cos(np.pi * np.arange(D))            # (-1)^d
    spec_q = np.fft.rfft(np.asarray(inputs["bq"], np.float64))
    spec_k = np.fft.rfft(np.asarray(inputs["bk"], np.float64))
    spec_v = np.fft.rfft(np.asarray(inputs["bv"], np.float64))
    w64 = {x: np.asarray(inputs[f"W{x}"], np.float64) for x in "qkv"}
    corr = np.zeros((R, D), np.float64)
    for bin_idx, fold in ((0, np.ones(D)), (FP, sgn)):
        gq = w64["q"].T @ fold
        gk = w64["k"].T @ fold
        gv = w64["v"].T @ fold
        qb = h64 @ gq + (spec_q.real[bin_idx])
        kb = h64 @ gk + (spec_k.real[bin_idx])
        vb = h64 @ gv + (spec_v.real[bin_idx])
        kb = kb / (np.abs(kb) + EPS)
        vb = vb / (np.abs(vb) + EPS)
        mem = np.cumsum((kb * vb).reshape(B, S), axis=1).reshape(R)
        ob = qb * mem / D                          # w=1 for real bins
        corr += np.outer(ob, fold)
    gate64 = float(np.asarray(inputs["gate"], np.float64).reshape(-1)[0])

    in_maps = []
    for c in range(NCORES):
        sl = slice(c * 2 * FPC, (c + 1) * 2 * FPC)
        im = {
            "h1": h1, "h2": h2,
            "ab1": ab1s[c], "ab2": ab2s[c],
            "dsc": descale,
            "bfc": np.ascontiguousarray(bfc[c * FPC:(c + 1) * FPC]),
        }
        for x in "kvq":
            im[f"g1{x}"] = np.ascontiguousarray(gf1[x][:, sl])
            im[f"g2{x}"] = np.ascontiguousarray(gf2[x][:, sl])
        in_maps.append(im)

    res = bass_utils.run_bass_kernel_spmd(
        nc, in_maps, core_ids=list(range(NCORES)), trace=trace)
    adapter = np.concatenate(
        [np.asarray(res.results[c]["out"]) for c in range(NCORES)], axis=0)
    full = np.ascontiguousarray(adapter.T).astype(np.float32)
    full += base + (gate64 * corr).astype(np.float32)
    return full.reshape(B, S, D), res


def kernel(**inputs) -> np.ndarray:
    out, _ = _run(inputs, trace=False)
    return out
